# revision 3
# baseline (speedup 1.0000x reference)
# Lagrangian-NN qddot kernel for TRN2 (8 NeuronCores, data-parallel over batch).
#
# Math: scalar L(q,qdot) = MLP(24->256x4->1, softplus). Per sample:
#   M = d2L/dqdot2 + 0.01 I ; C = d2L/dqdot dq ; qddot = M^-1 (dL/dq - C qdot).
# Batched fwd+bwd gives grad; 12 qdot-direction forward-over-reverse tangents give
# H[:,12:] whose symmetry supplies both M and the Coriolis contraction; per-sample
# 12x12 solve by batched no-pivot Gauss-Jordan (M ~ 0.01*I, cond ~1.06).
# softplus/sigmoid composed from {abs,exp,ln,relu} (single ACT table set):
#   Z = relu(x) + ln(exp(-|x|)+1) ; S = exp(x - Z).
import os
import sys
import numpy as np

for p in ("/opt/trn_rl_repo", "/root/.axon_site/_ro/trn_rl_repo"):
    if p not in sys.path:
        sys.path.insert(0, p)

import concourse.bass as bass
import concourse.mybir as mybir
import concourse.tile as tile
from concourse import bacc
from concourse.bass_utils import run_bass_kernel_spmd

F32 = mybir.dt.float32
F32R = mybir.dt.float32r
AF = mybir.ActivationFunctionType
ALU = mybir.AluOpType

B, ND, H, NC = 8192, 12, 256, 8
N = B // NC
IN = 2 * ND
T = 64
NT = N // T
NG = N // 128
FD = ND * T
CH = 512
KT = H // 128

_cache = {}


def build_kernel():
    nc = bacc.Bacc("TRN2", target_bir_lowering=False)
    dq = nc.dram_tensor("q", (N, ND), F32, kind="ExternalInput")
    dqd = nc.dram_tensor("qdot", (N, ND), F32, kind="ExternalInput")
    dWT = [nc.dram_tensor(f"WT{l}", s, F32, kind="ExternalInput")
           for l, s in enumerate([(IN, H), (H, H), (H, H), (H, H)])]
    dWn = {l: nc.dram_tensor(f"Wn{l}", (H, H), F32, kind="ExternalInput") for l in (1, 2, 3)}
    dW0 = nc.dram_tensor("W0n", (H, IN), F32, kind="ExternalInput")
    dbs = [nc.dram_tensor(f"b{l}", (H, 1), F32, kind="ExternalInput") for l in range(4)]
    dw4 = nc.dram_tensor("w4", (H, 1), F32, kind="ExternalInput")
    dide = nc.dram_tensor("ident", (128, 128), F32, kind="ExternalInput")
    dout = nc.dram_tensor("qdd", (N, ND), F32, kind="ExternalOutput")

    with tile.TileContext(nc) as tc:
        with tc.tile_pool(name="wp", bufs=1) as wp, \
             tc.tile_pool(name="acts", bufs=1) as actp, \
             tc.tile_pool(name="tang", bufs=1) as tgp, \
             tc.tile_pool(name="scr", bufs=1) as scr, \
             tc.tile_pool(name="psB", bufs=2, space="PSUM") as psB, \
             tc.tile_pool(name="psT", bufs=2, space="PSUM") as psT:

            ident = wp.tile([128, 128], F32)
            nc.sync.dma_start(ident[:], dide[:])

            def load_round(dram, P, Fr, tag):
                parts = []
                for ki, p0 in enumerate(range(0, P, 128)):
                    pe = min(P, p0 + 128)
                    raw = scr.tile([pe - p0, Fr], F32, tag="wraw")
                    nc.sync.dma_start(raw[:], dram[p0:pe, :])
                    r = wp.tile([pe - p0, Fr], F32R, tag=f"{tag}_{ki}")
                    nc.scalar.activation(r[:], raw[:], AF.Copy)
                    parts.append(r)
                return parts

            WT_r = [load_round(dWT[l], (IN if l == 0 else H), H, f"WT{l}") for l in range(4)]
            Wn_r = {l: load_round(dWn[l], H, H, f"Wn{l}") for l in (1, 2, 3)}
            W0_r = load_round(dW0, H, IN, "W0n")
            W0q = []
            for ki in range(KT):
                t = wp.tile([128, ND], F32, tag=f"W0q{ki}")
                nc.sync.dma_start(t[:], dW0[ki * 128:(ki + 1) * 128, ND:])
                W0q.append(t)
            bs = []
            for l in range(4):
                ps_ = []
                for ki in range(KT):
                    t = wp.tile([128, 1], F32, tag=f"b{l}_{ki}")
                    nc.sync.dma_start(t[:], dbs[l][ki * 128:(ki + 1) * 128, :])
                    ps_.append(t)
                bs.append(ps_)
            w4t = []
            for ki in range(KT):
                t = wp.tile([128, 1], F32, tag=f"w4_{ki}")
                nc.sync.dma_start(t[:], dw4[ki * 128:(ki + 1) * 128, :])
                w4t.append(t)

            xts = []
            XT = actp.tile([IN, N], F32R)
            for g in range(NG):
                xt = actp.tile([128, IN], F32, tag=f"xt{g}")
                nc.sync.dma_start(xt[:, 0:ND], dq[g * 128:(g + 1) * 128, :])
                nc.sync.dma_start(xt[:, ND:], dqd[g * 128:(g + 1) * 128, :])
                xts.append(xt)
                pt = psT.tile([IN, 128], F32, tag="ptx")
                nc.tensor.transpose(pt[:], xt[:], ident[:])
                nc.scalar.activation(XT[:, g * 128:(g + 1) * 128], pt[:], AF.Copy)

            def mm(psum_ap, lhsT_parts, rhs_parts, Fr):
                nk = len(lhsT_parts)
                for c0 in range(0, Fr, CH):
                    ce = min(Fr, c0 + CH)
                    for ki in range(nk):
                        nc.tensor.matmul(psum_ap[:, c0:ce], lhsT_parts[ki],
                                         rhs_parts[ki][:, c0:ce],
                                         start=(ki == 0), stop=(ki == nk - 1))

            def fwd_layer(rhs_parts, Wparts, K, lidx):
                Zs, Ss = [], []
                nk = (K + 127) // 128
                for ot in range(KT):
                    ps = psB.tile([128, 1024], F32, tag="big")
                    lts = [Wparts[k][:, ot * 128:(ot + 1) * 128] for k in range(nk)]
                    mm(ps[:, 0:N], lts, rhs_parts, N)
                    Ap = scr.tile([128, N], F32, tag="Ap")
                    nc.scalar.activation(Ap[:], ps[0:128, 0:N], AF.Identity,
                                         bias=bs[lidx][ot][:])
                    ab = scr.tile([128, N], F32, tag="ab")
                    nc.scalar.activation(ab[:], Ap[:], AF.Abs)
                    ex = scr.tile([128, N], F32, tag="ex")
                    nc.scalar.activation(ex[:], ab[:], AF.Exp, scale=-1.0)
                    ln = scr.tile([128, N], F32, tag="ln")
                    nc.scalar.activation(ln[:], ex[:], AF.Ln, bias=1.0)
                    rl = scr.tile([128, N], F32, tag="rl")
                    nc.scalar.activation(rl[:], Ap[:], AF.Relu)
                    Z = actp.tile([128, N], F32R, tag=f"Zf{lidx % 2}_{ot}")
                    nc.vector.tensor_add(Z[:], rl[:], ln[:])
                    d = scr.tile([128, N], F32, tag="d")
                    nc.vector.tensor_sub(d[:], Ap[:], Z[:].bitcast(F32))
                    S = actp.tile([128, N], F32, tag=f"S{lidx}_{ot}")
                    nc.scalar.activation(S[:], d[:], AF.Exp)
                    Zs.append(Z)
                    Ss.append(S)
                return Zs, Ss

            Z1, S1 = fwd_layer([XT[:]], WT_r[0], IN, 0)
            Z2, S2 = fwd_layer([z[:] for z in Z1], WT_r[1], H, 1)
            Z3, S3 = fwd_layer([z[:] for z in Z2], WT_r[2], H, 2)
            _, S4 = fwd_layer([z[:] for z in Z3], WT_r[3], H, 3)

            D4, c4 = [], []
            for ot in range(KT):
                D = actp.tile([128, N], F32R, tag=f"Dr0_{ot}")
                nc.vector.tensor_scalar_mul(D[:], S4[ot][:], w4t[ot][:])
                D4.append(D)
                t1 = scr.tile([128, N], F32, tag="c4t")
                nc.vector.tensor_mul(t1[:], D[:].bitcast(F32), S4[ot][:])
                c = actp.tile([128, N], F32, tag=f"c4_{ot}")
                nc.vector.tensor_sub(c[:], D[:].bitcast(F32), t1[:])
                c4.append(c)

            def bwd_layer(Dup, Wparts, Sl, lidx, want_F):
                Ds, Fs = [], []
                for ot in range(KT):
                    ps = psB.tile([128, 1024], F32, tag="big")
                    lts = [Wparts[k][:, ot * 128:(ot + 1) * 128] for k in range(KT)]
                    mm(ps[:, 0:N], lts, [d[:] for d in Dup], N)
                    D = actp.tile([128, N], F32R, tag=f"Dr{lidx % 2}_{ot}")
                    nc.vector.tensor_mul(D[:], Sl[ot][:], ps[0:128, 0:N])
                    Ds.append(D)
                    if want_F:
                        F = actp.tile([128, N], F32, tag=f"F{lidx}_{ot}")
                        nc.vector.tensor_sub(F[:], ps[0:128, 0:N], D[:].bitcast(F32))
                        Fs.append(F)
                return Ds, Fs

            D3, F3 = bwd_layer(D4, Wn_r[3], S3, 3, True)
            D2, F2 = bwd_layer(D3, Wn_r[2], S2, 2, True)
            D1, _ = bwd_layer(D2, Wn_r[1], S1, 1, False)
            E1 = []
            for ot in range(KT):
                t1 = scr.tile([128, N], F32, tag="e1t")
                nc.vector.tensor_mul(t1[:], D1[ot][:].bitcast(F32), S1[ot][:])
                E = actp.tile([128, N], F32, tag=f"E1_{ot}")
                nc.vector.tensor_sub(E[:], D1[ot][:].bitcast(F32), t1[:])
                E1.append(E)

            Gps = psB.tile([128, 1024], F32, tag="big")
            mm(Gps[0:IN, 0:N], [W0_r[k][:] for k in range(KT)], [d[:] for d in D1], N)
            G = actp.tile([IN, N], F32)
            nc.scalar.activation(G[:], Gps[0:IN, 0:N], AF.Copy)

            aug = actp.tile([128, 13 * ND * NG], F32)
            aug4 = aug[:].rearrange("p (i j g) -> p i j g", i=ND, j=13, g=NG)

            for g in range(NG):
                ptg = psT.tile([128, IN], F32, tag="ptx")
                nc.tensor.transpose(ptg[0:128, 0:ND], G[0:ND, g * 128:(g + 1) * 128],
                                    ident[0:ND, 0:ND])
                nc.scalar.activation(aug4[:, :, 12, g], ptg[0:128, 0:ND], AF.Copy)

            Hc128 = CHq = None
            for b in range(NT):
                g, off = b // 2, (b % 2) * 64
                sl = slice(b * T, (b + 1) * T)

                def bca(Sten, ot):
                    return Sten[ot][:, sl].unsqueeze(1).broadcast_to((128, ND, T))

                def t3d(ps):
                    return ps[0:128, 0:FD].rearrange("p (d t) -> p d t", d=ND)

                Zd1 = []
                for ot in range(KT):
                    z = tgp.tile([128, ND, T], F32R, tag=f"ZdA_{ot}")
                    wq = W0q[ot][:].unsqueeze(2).broadcast_to((128, ND, T))
                    nc.vector.tensor_tensor(z[:], bca(S1, ot), wq, ALU.mult)
                    Zd1.append(z)

                def tang_fwd(Zin, Wparts, Sl, ztag):
                    outs = []
                    for ot in range(KT):
                        ps = psB.tile([128, 1024], F32, tag="big")
                        lts = [Wparts[k][:, ot * 128:(ot + 1) * 128] for k in range(KT)]
                        mm(ps[:, 0:FD], lts,
                           [z[:].rearrange("p d t -> p (d t)") for z in Zin], FD)
                        z = tgp.tile([128, ND, T], F32R, tag=f"Zd{ztag}_{ot}")
                        nc.vector.tensor_tensor(z[:], bca(Sl, ot), t3d(ps), ALU.mult)
                        outs.append(z)
                    return outs

                Zd2 = tang_fwd(Zd1, WT_r[1], S2, "B")
                Zd3 = tang_fwd(Zd2, WT_r[2], S3, "C")

                Dd4 = []
                for ot in range(KT):
                    ps = psB.tile([128, 1024], F32, tag="big")
                    lts = [WT_r[3][k][:, ot * 128:(ot + 1) * 128] for k in range(KT)]
                    mm(ps[:, 0:FD], lts,
                       [z[:].rearrange("p d t -> p (d t)") for z in Zd3], FD)
                    dd = tgp.tile([128, ND, T], F32R, tag=f"DdA_{ot}")
                    nc.vector.tensor_tensor(dd[:], bca(c4, ot), t3d(ps), ALU.mult)
                    Dd4.append(dd)

                def tang_bwd(Ddup, Wparts, Sl, Fl, Zdl, dtag):
                    outs = []
                    for ot in range(KT):
                        ps = psB.tile([128, 1024], F32, tag="big")
                        lts = [Wparts[k][:, ot * 128:(ot + 1) * 128] for k in range(KT)]
                        mm(ps[:, 0:FD], lts,
                           [d[:].rearrange("p d t -> p (d t)") for d in Ddup], FD)
                        tb = scr.tile([128, ND, T], F32, tag="tB")
                        nc.vector.tensor_tensor(tb[:], bca(Sl, ot), t3d(ps), ALU.mult)
                        ta = scr.tile([128, ND, T], F32, tag="tA2")
                        nc.vector.tensor_tensor(ta[:], bca(Fl, ot),
                                                Zdl[ot][:].bitcast(F32), ALU.mult)
                        dd = tgp.tile([128, ND, T], F32R, tag=f"Dd{dtag}_{ot}")
                        nc.gpsimd.tensor_add(dd[:].rearrange("p d t -> p (d t)"),
                                             ta[:].rearrange("p d t -> p (d t)"),
                                             tb[:].rearrange("p d t -> p (d t)"))
                        outs.append(dd)
                    return outs

                Dd3 = tang_bwd(Dd4, Wn_r[3], S3, F3, Zd3, "B")
                Dd2 = tang_bwd(Dd3, Wn_r[2], S2, F2, Zd2, "A")

                Dd1 = []
                for ot in range(KT):
                    ps = psB.tile([128, 1024], F32, tag="big")
                    lts = [Wn_r[1][k][:, ot * 128:(ot + 1) * 128] for k in range(KT)]
                    mm(ps[:, 0:FD], lts,
                       [d[:].rearrange("p d t -> p (d t)") for d in Dd2], FD)
                    tb = scr.tile([128, ND, T], F32, tag="tB")
                    nc.vector.tensor_tensor(tb[:], bca(S1, ot), t3d(ps), ALU.mult)
                    ta = scr.tile([128, ND, T], F32, tag="tA2")
                    wq = W0q[ot][:].unsqueeze(2).broadcast_to((128, ND, T))
                    nc.vector.tensor_tensor(ta[:], bca(E1, ot), wq, ALU.mult)
                    dd = tgp.tile([128, ND, T], F32R, tag=f"DdB_{ot}")
                    nc.gpsimd.tensor_add(dd[:].rearrange("p d t -> p (d t)"),
                                         ta[:].rearrange("p d t -> p (d t)"),
                                         tb[:].rearrange("p d t -> p (d t)"))
                    Dd1.append(dd)

                psH = psB.tile([128, 1024], F32, tag="big")
                mm(psH[0:IN, 0:FD], [W0_r[k][:] for k in range(KT)],
                   [d[:].rearrange("p d t -> p (d t)") for d in Dd1], FD)
                if off == 0:
                    Hc128 = scr.tile([IN, ND, 128], F32, tag="Hc")
                    CHq = scr.tile([128, ND * ND], F32, tag="CHq")
                nc.scalar.activation(Hc128[:, :, off:off + T],
                                     psH[0:IN, 0:FD].rearrange("p (d t) -> p d t", d=ND),
                                     AF.Copy)

                if off == 64:
                    for d in range(ND):
                        pt = psT.tile([128, IN], F32, tag="ptH")
                        nc.tensor.transpose(pt[:], Hc128[:, d, :], ident[0:IN, 0:IN])
                        nc.scalar.activation(CHq[:, d * ND:(d + 1) * ND],
                                             pt[:, 0:ND], AF.Copy)
                        nc.scalar.activation(aug4[:, :, d, g], pt[:, ND:IN], AF.Copy)
                    prod = scr.tile([128, ND, ND], F32, tag="prod")
                    qdv = xts[g][:, ND:IN].unsqueeze(1).broadcast_to((128, ND, ND))
                    nc.vector.tensor_tensor(prod[:], CHq[:].rearrange("p (i j) -> p i j", j=ND),
                                            qdv, ALU.mult)
                    cor = scr.tile([128, ND], F32, tag="cor")
                    nc.vector.tensor_reduce(cor[:].unsqueeze(2), prod[:], op=ALU.add,
                                            axis=mybir.AxisListType.X)
                    nc.vector.tensor_sub(aug4[:, :, 12, g], aug4[:, :, 12, g], cor[:])

            for i in range(ND):
                nc.vector.tensor_scalar_add(aug4[:, i, i, :], aug4[:, i, i, :], 0.01)

            for k in range(ND):
                piv = aug4[:, k, k, :]
                rec = scr.tile([128, NG], F32, tag="rec")
                nc.vector.reciprocal(rec[:], piv)
                nw = 12 - k
                rk = aug4[:, k, k + 1:13, :]
                recb = rec[:].unsqueeze(1).broadcast_to((128, nw, NG))
                nc.vector.scalar_tensor_tensor(rk, rk, -1.0, recb, ALU.mult, ALU.mult)
                for i in range(ND):
                    if i == k:
                        continue
                    fb = aug4[:, i, k, :].unsqueeze(1).broadcast_to((128, nw, NG))
                    tmv = scr.tile([128, nw, NG], F32, tag="gjt")
                    nc.vector.tensor_tensor(tmv[:], rk, fb, ALU.mult)
                    nc.vector.tensor_add(aug4[:, i, k + 1:13, :], aug4[:, i, k + 1:13, :], tmv[:])

            for g in range(NG):
                xo = scr.tile([128, ND], F32, tag="xo")
                nc.vector.tensor_scalar_mul(xo[:], aug4[:, :, 12, g], -1.0)
                nc.sync.dma_start(dout[g * 128:(g + 1) * 128, :], xo[:])

    nc.compile()
    return nc


def kernel(**inputs):
    q = np.ascontiguousarray(inputs["q"], dtype=np.float32)
    qdot = np.ascontiguousarray(inputs["qdot"], dtype=np.float32)
    if "nc" not in _cache:
        _cache["nc"] = build_kernel()
    nc = _cache["nc"]
    base = {
        "WT0": np.ascontiguousarray(inputs["W0"].T).astype(np.float32),
        "WT1": np.ascontiguousarray(inputs["W1"].T).astype(np.float32),
        "WT2": np.ascontiguousarray(inputs["W2"].T).astype(np.float32),
        "WT3": np.ascontiguousarray(inputs["W3"].T).astype(np.float32),
        "Wn1": np.ascontiguousarray(inputs["W1"]).astype(np.float32),
        "Wn2": np.ascontiguousarray(inputs["W2"]).astype(np.float32),
        "Wn3": np.ascontiguousarray(inputs["W3"]).astype(np.float32),
        "W0n": np.ascontiguousarray(inputs["W0"]).astype(np.float32),
        "b0": inputs["b0"].reshape(H, 1).astype(np.float32),
        "b1": inputs["b1"].reshape(H, 1).astype(np.float32),
        "b2": inputs["b2"].reshape(H, 1).astype(np.float32),
        "b3": inputs["b3"].reshape(H, 1).astype(np.float32),
        "w4": np.ascontiguousarray(inputs["W4"].reshape(H, 1)).astype(np.float32),
        "ident": np.eye(128, dtype=np.float32),
    }
    in_maps = []
    for c in range(NC):
        m = dict(base)
        m["q"] = q[c * N:(c + 1) * N]
        m["qdot"] = qdot[c * N:(c + 1) * N]
        in_maps.append(m)
    res = run_bass_kernel_spmd(nc, in_maps, core_ids=list(range(NC)),
                               trace=bool(os.environ.get("LNN_TRACE")))
    _cache["last"] = res
    out = np.concatenate([res.results[c]["qdd"] for c in range(NC)], axis=0)
    return out.astype(np.float32)



# revision 11
# speedup vs baseline: 1.0929x; 1.0929x over previous
# Lagrangian-NN qddot kernel for TRN2 (8 NeuronCores, data-parallel over batch).
#
# Math: scalar L(q,qdot) = MLP(24->256x4->1, softplus). Per sample:
#   M = d2L/dqdot2 + 0.01 I ; C = d2L/dqdot dq ; qddot = M^-1 (dL/dq - C qdot).
# Batched fwd+bwd gives grad; 12 qdot-direction forward-over-reverse tangents give
# H[:,12:] whose symmetry supplies both M and the Coriolis contraction.
# Everything except PSUM accumulation and the H/solve stage runs in fp16:
#  - DVE elementwise ops are emitted as scalar_tensor_tensor/tensor_scalar
#    (InstTensorScalarPtr) which support the 2x/4x DVE perf modes on all-SBUF
#    2-byte operands; PSUM results are staged to fp16 SBUF via Act/Pool copies.
#  - softplus/sigmoid composed from {exp,ln} + DVE max/adds (single ACT table):
#    Z = max(A,0) + ln(exp(-|A|)+1) ; S = exp(A - Z).
#  - M = 0.01(I + 100*Hqd) with ||100*Hqd|| <= 0.035, so the 12x12 solve is a
#    3-term Neumann series, fused over all 8 sample groups per core.
import os
import sys
import numpy as np

for p in ("/opt/trn_rl_repo", "/root/.axon_site/_ro/trn_rl_repo"):
    if p not in sys.path:
        sys.path.insert(0, p)

import concourse.bass as bass
import concourse.mybir as mybir
import concourse.tile as tile
from concourse import bacc
from concourse.bass_utils import run_bass_kernel_spmd

F32 = mybir.dt.float32
F16 = mybir.dt.float16
AF = mybir.ActivationFunctionType
ALU = mybir.AluOpType
AX = mybir.AxisListType

B, ND, H, NC = 8192, 12, 256, 8
N = B // NC          # samples per core
IN = 2 * ND          # 24
T = 64               # samples per tangent block
NT = N // T          # 16 blocks
NG = N // 128        # 8 groups of 128 samples
FD = ND * T          # 768 tangent free dim
CH = 512             # psum bank chunk (fp32 cols)
KT = H // 128        # 2 k-tiles per hidden dim

_cache = {}


def build_kernel():
    nc = bacc.Bacc("TRN2", target_bir_lowering=False)
    dx16 = nc.dram_tensor("x16", (N, IN), F16, kind="ExternalInput")
    dqd = nc.dram_tensor("qd32", (N, ND), F32, kind="ExternalInput")
    dwt0 = nc.dram_tensor("wt0", (IN, H), F16, kind="ExternalInput")
    dwt = {l: nc.dram_tensor(f"wt{l}", (H, H), F16, kind="ExternalInput")
           for l in (1, 2, 3)}
    dwn = {l: nc.dram_tensor(f"wn{l}", (H, H), F16, kind="ExternalInput")
           for l in (1, 2, 3)}
    dw0n = nc.dram_tensor("w0n", (H, IN), F16, kind="ExternalInput")
    dw0qr = nc.dram_tensor("w0qr", (H, FD), F16, kind="ExternalInput")
    dbs = [nc.dram_tensor(f"b{l}", (H, 1), F32, kind="ExternalInput") for l in range(4)]
    dw4 = nc.dram_tensor("w4", (H, 1), F32, kind="ExternalInput")
    did16 = nc.dram_tensor("id16", (128, 128), F16, kind="ExternalInput")
    did32 = nc.dram_tensor("id32", (128, 128), F32, kind="ExternalInput")
    dout = nc.dram_tensor("qdd", (N, ND), F32, kind="ExternalOutput")

    with tile.TileContext(nc) as tc:
        with tc.tile_pool(name="wp", bufs=1) as wp, \
             tc.tile_pool(name="ap", bufs=1) as ap, \
             tc.tile_pool(name="sc", bufs=2) as sc, \
             tc.tile_pool(name="tg", bufs=2) as tg, \
             tc.tile_pool(name="hp", bufs=1) as hp, \
             tc.tile_pool(name="psB", bufs=3, space="PSUM") as psB, \
             tc.tile_pool(name="psT", bufs=2, space="PSUM") as psT:

            # ---- weight / const loads --------------------------------------
            id16 = wp.tile([128, 128], F16)
            nc.sync.dma_start(id16[:], did16[:])
            id32 = wp.tile([128, 128], F32)
            nc.sync.dma_start(id32[:], did32[:])

            WT0 = wp.tile([IN, H], F16)
            nc.sync.dma_start(WT0[:], dwt0[:])

            def load2(dram, Fr, tag, dt=F16):
                parts = []
                for ki in range(KT):
                    t_ = wp.tile([128, Fr], dt, tag=f"{tag}{ki}")
                    nc.sync.dma_start(t_[:], dram[ki * 128:(ki + 1) * 128, :])
                    parts.append(t_)
                return parts

            WT = {l: load2(dwt[l], H, f"wt{l}_") for l in (1, 2, 3)}
            Wn = {l: load2(dwn[l], H, f"wn{l}_") for l in (1, 2, 3)}
            W0n = load2(dw0n, IN, "w0n_")
            W0qr = load2(dw0qr, FD, "w0qr_")
            bs = []
            for l in range(4):
                row = []
                for ki in range(KT):
                    t_ = wp.tile([128, 1], F32, tag=f"b{l}_{ki}")
                    nc.sync.dma_start(t_[:], dbs[l][ki * 128:(ki + 1) * 128, :])
                    row.append(t_)
                bs.append(row)
            w4t = []
            for ki in range(KT):
                t_ = wp.tile([128, 1], F32, tag=f"w4_{ki}")
                nc.sync.dma_start(t_[:], dw4[ki * 128:(ki + 1) * 128, :])
                w4t.append(t_)

            XS = hp.tile([128, NG, IN], F16)
            qd_all = hp.tile([128, NG, ND], F32)
            for g in range(NG):
                nc.sync.dma_start(XS[:, g, :], dx16[g * 128:(g + 1) * 128, :])
                nc.sync.dma_start(qd_all[:, g, :], dqd[g * 128:(g + 1) * 128, :])

            # ---- XT = X^T [24, N] fp16 -------------------------------------
            XT = hp.tile([IN, N], F16)
            for g in range(NG):
                pt = psT.tile([IN, 128], F16, tag="pt")
                nc.tensor.transpose(pt[:], XS[:, g, :], id16[:])
                nc.vector.tensor_copy(XT[:, g * 128:(g + 1) * 128], pt[:])

            def mm(ps_ap, lhsT_list, rhs_list, Fr):
                nk = len(lhsT_list)
                for c0 in range(0, Fr, CH):
                    ce = min(Fr, c0 + CH)
                    for ki in range(nk):
                        nc.tensor.matmul(ps_ap[:, c0:ce], lhsT_list[ki],
                                         rhs_list[ki][:, c0:ce],
                                         start=(ki == 0), stop=(ki == nk - 1))

            # ---- forward: Z chain + S (softplus/sigmoid via exp/ln) --------
            S = {}
            Zprev = [XT[:]]
            lhs0 = [WT0[:]]
            for l in range(4):
                Zcur = []
                for ot in range(KT):
                    ps = psB.tile([128, 1024], F32, tag="mm")
                    if l == 0:
                        mm(ps[:, 0:N], [lhs0[0][:, ot * 128:(ot + 1) * 128]],
                           Zprev, N)
                    else:
                        lts = [WT[l][ki][:, ot * 128:(ot + 1) * 128] for ki in range(KT)]
                        mm(ps[:, 0:N], lts, Zprev, N)
                    A16 = sc.tile([128, N], F16, tag="A16")
                    nc.scalar.activation(A16[:], ps[0:128, 0:N], AF.Identity,
                                         bias=bs[l][ot][:])
                    ab = sc.tile([128, N], F16, tag="t1")
                    nc.vector.scalar_tensor_tensor(ab[:], A16[:], -1.0, A16[:],
                                                   ALU.mult, ALU.max)
                    ex = sc.tile([128, N], F16, tag="t2")
                    nc.scalar.activation(ex[:], ab[:], AF.Exp, scale=-1.0)
                    Ln = sc.tile([128, N], F16, tag="L")
                    nc.scalar.activation(Ln[:], ex[:], AF.Ln, bias=1.0)
                    rl = sc.tile([128, N], F16, tag="t1")
                    nc.vector.tensor_scalar_max(rl[:], A16[:], 0.0)
                    Z = sc.tile([128, N], F16, tag="Z", bufs=3)
                    nc.vector.scalar_tensor_tensor(Z[:], rl[:], 1.0, Ln[:],
                                                   ALU.mult, ALU.add)
                    d = sc.tile([128, N], F16, tag="t2")
                    nc.vector.scalar_tensor_tensor(d[:], Z[:], -1.0, A16[:],
                                                   ALU.mult, ALU.add)
                    St = ap.tile([128, N], F16, tag=f"S{l}_{ot}")
                    nc.scalar.activation(St[:], d[:], AF.Exp)
                    S[(l, ot)] = St
                    Zcur.append(Z)
                Zprev = [z[:] for z in Zcur]

            # ---- D4 / c4 ----------------------------------------------------
            D4, c4 = [], []
            for ot in range(KT):
                Dt4 = ap.tile([128, N], F16, tag=f"D4_{ot}")
                nc.vector.tensor_scalar_mul(Dt4[:], S[(3, ot)][:], w4t[ot][:])
                D4.append(Dt4)
                OmS = sc.tile([128, N], F16, tag="OmS")
                nc.vector.tensor_scalar(OmS[:], S[(3, ot)][:], -1.0, 1.0,
                                        ALU.mult, ALU.add)
                ct = ap.tile([128, N], F16, tag=f"c4_{ot}")
                nc.vector.scalar_tensor_tensor(ct[:], OmS[:], w4t[ot][:], S[(3, ot)][:],
                                               ALU.mult, ALU.mult)
                c4.append(ct)

            # ---- backward D chain + F/E + g_q ------------------------------
            F = {}
            E1 = []
            Dprev = [d[:] for d in D4]
            for l in (2, 1, 0):
                Dcur = []
                for ot in range(KT):
                    ps = psB.tile([128, 1024], F32, tag="mm")
                    lts = [Wn[l + 1][ki][:, ot * 128:(ot + 1) * 128] for ki in range(KT)]
                    mm(ps[:, 0:N], lts, Dprev, N)
                    Ucp = sc.tile([128, N], F16, tag="Ucp")
                    nc.scalar.activation(Ucp[:], ps[0:128, 0:N], AF.Copy)
                    Dt = sc.tile([128, N], F16, tag="Dt", bufs=3)
                    nc.vector.scalar_tensor_tensor(Dt[:], Ucp[:], 1.0, S[(l, ot)][:],
                                                   ALU.mult, ALU.mult)
                    if l > 0:
                        Ft = ap.tile([128, N], F16, tag=f"F{l}_{ot}")
                        nc.vector.scalar_tensor_tensor(Ft[:], Dt[:], -1.0, Ucp[:],
                                                       ALU.mult, ALU.add)
                        F[(l, ot)] = Ft
                    else:
                        OmS1 = sc.tile([128, N], F16, tag="OmS")
                        nc.vector.tensor_scalar(OmS1[:], S[(0, ot)][:], -1.0, 1.0,
                                                ALU.mult, ALU.add)
                        Et = ap.tile([128, N], F16, tag=f"E1_{ot}")
                        nc.vector.scalar_tensor_tensor(Et[:], Dt[:], 1.0, OmS1[:],
                                                       ALU.mult, ALU.mult)
                        E1.append(Et)
                    Dcur.append(Dt)
                Dprev = [d[:] for d in Dcur]

            psG = psB.tile([128, 1024], F32, tag="mm")
            mm(psG[0:IN, 0:N], [W0n[ki][:] for ki in range(KT)], Dprev, N)
            Gcp = hp.tile([IN, N], F32)
            nc.scalar.activation(Gcp[:], psG[0:IN, 0:N], AF.Copy)

            # transposed g_q per group -> gqT [128, g, 12]
            gqT = hp.tile([128, NG, ND], F32)
            for g in range(NG):
                ptg = psT.tile([128, ND], F32, tag="pt")
                nc.tensor.transpose(ptg[:], Gcp[0:ND, g * 128:(g + 1) * 128],
                                    id32[0:ND, 0:ND])
                nc.vector.tensor_copy(gqT[:, g, :], ptg[:])

            # ---- tangent blocks --------------------------------------------
            Hq = hp.tile([128, NG, ND, ND], F32)   # H[x_j<12, qd_i] -> [p,g,i,j]
            Hm = hp.tile([128, NG, ND, ND], F32)   # 100*H[x_12+j, qd_i]
            Hc = None

            def bca(l, ot, sl):
                return S[(l, ot)][:, sl].unsqueeze(1).broadcast_to((128, ND, T))

            for b in range(NT):
                g, off = b // 2, (b % 2) * T
                sl = slice(b * T, (b + 1) * T)

                Zd1 = []
                for ot in range(KT):
                    z = tg.tile([128, ND, T], F16, tag=f"Zd1_{ot}")
                    w0v = W0qr[ot][:].rearrange("p (d t) -> p d t", d=ND)
                    nc.vector.scalar_tensor_tensor(z[:], w0v, 1.0, bca(0, ot, sl),
                                                   ALU.mult, ALU.mult)
                    Zd1.append(z)

                def flat(ts):
                    return [t_[:].rearrange("p d t -> p (d t)") for t_ in ts]

                def tang_fwd(Zin, lW, Sl, Scoef, ztag, copy_eng):
                    outs = []
                    for ot in range(KT):
                        ps = psB.tile([128, 1024], F32, tag="mm")
                        lts = [lW[ki][:, ot * 128:(ot + 1) * 128] for ki in range(KT)]
                        mm(ps[:, 0:FD], lts, flat(Zin), FD)
                        cc = tg.tile([128, ND, T], F16, tag=f"c_{ot}", bufs=3)
                        ccf = cc[:].rearrange("p d t -> p (d t)")
                        nc.scalar.activation(ccf, ps[0:128, 0:FD], AF.Copy)
                        z = tg.tile([128, ND, T], F16, tag=f"{ztag}_{ot}")
                        nc.vector.scalar_tensor_tensor(
                            z[:], cc[:], 1.0,
                            Scoef[ot][:, sl].unsqueeze(1).broadcast_to((128, ND, T)),
                            ALU.mult, ALU.mult)
                        outs.append(z)
                    return outs

                Zd2 = tang_fwd(Zd1, WT[1], None, [S[(1, 0)], S[(1, 1)]], "Zd2", "act")
                Zd3 = tang_fwd(Zd2, WT[2], None, [S[(2, 0)], S[(2, 1)]], "Zd3", "pool")
                Dd4 = tang_fwd(Zd3, WT[3], None, c4, "Dd4", "pool")

                def tang_bwd(Ddup, lW, Sl_l, Zdl, Fl, dtag, copy_eng):
                    outs = []
                    for ot in range(KT):
                        ps = psB.tile([128, 1024], F32, tag="mm")
                        lts = [lW[ki][:, ot * 128:(ot + 1) * 128] for ki in range(KT)]
                        mm(ps[:, 0:FD], lts, flat(Ddup), FD)
                        y = tg.tile([128, ND, T], F16, tag=f"y_{ot}", bufs=3)
                        yf = y[:].rearrange("p d t -> p (d t)")
                        nc.scalar.activation(yf, ps[0:128, 0:FD], AF.Copy)
                        u = tg.tile([128, ND, T], F16, tag=f"u_{ot}")
                        nc.vector.scalar_tensor_tensor(u[:], y[:], 1.0, bca(Sl_l, ot, sl),
                                                       ALU.mult, ALU.mult)
                        t_ = tg.tile([128, ND, T], F16, tag=f"t_{ot}")
                        nc.vector.scalar_tensor_tensor(
                            t_[:], Zdl[ot][:], 1.0,
                            Fl[ot][:, sl].unsqueeze(1).broadcast_to((128, ND, T)),
                            ALU.mult, ALU.mult)
                        dd = tg.tile([128, ND, T], F16, tag=f"{dtag}_{ot}")
                        nc.vector.scalar_tensor_tensor(dd[:], u[:], 1.0, t_[:],
                                                       ALU.mult, ALU.add)
                        outs.append(dd)
                    return outs

                Dd3 = tang_bwd(Dd4, Wn[3], 2, Zd3, [F[(2, 0)], F[(2, 1)]], "DdA", "act")
                Dd2 = tang_bwd(Dd3, Wn[2], 1, Zd2, [F[(1, 0)], F[(1, 1)]], "DdB", "pool")

                # layer 1: t = E1 * W0qr
                Dd1 = []
                for ot in range(KT):
                    ps = psB.tile([128, 1024], F32, tag="mm")
                    lts = [Wn[1][ki][:, ot * 128:(ot + 1) * 128] for ki in range(KT)]
                    mm(ps[:, 0:FD], lts, flat(Dd2), FD)
                    y = tg.tile([128, ND, T], F16, tag=f"y_{ot}", bufs=3)
                    nc.scalar.activation(y[:].rearrange("p d t -> p (d t)"),
                                         ps[0:128, 0:FD], AF.Copy)
                    u = tg.tile([128, ND, T], F16, tag=f"u_{ot}")
                    nc.vector.scalar_tensor_tensor(u[:], y[:], 1.0, bca(0, ot, sl),
                                                   ALU.mult, ALU.mult)
                    t_ = tg.tile([128, ND, T], F16, tag=f"t_{ot}")
                    w0v = W0qr[ot][:].rearrange("p (d t) -> p d t", d=ND)
                    nc.vector.scalar_tensor_tensor(
                        t_[:], w0v, 1.0,
                        E1[ot][:, sl].unsqueeze(1).broadcast_to((128, ND, T)),
                        ALU.mult, ALU.mult)
                    dd = tg.tile([128, ND, T], F16, tag=f"DdA_{ot}")
                    nc.vector.scalar_tensor_tensor(dd[:], u[:], 1.0, t_[:],
                                                   ALU.mult, ALU.add)
                    Dd1.append(dd)

                psH = psB.tile([128, 1024], F32, tag="mm")
                mm(psH[0:IN, 0:FD], [W0n[ki][:] for ki in range(KT)], flat(Dd1), FD)
                if off == 0:
                    Hc = hp.tile([IN, ND, 128], F16, tag="Hc", bufs=2)
                nc.scalar.activation(Hc[:, :, off:off + T],
                                     psH[0:IN, 0:FD].rearrange("p (d t) -> p d t", d=ND),
                                     AF.Copy)

                if off == T:
                    ptH = psT.tile([128, 288], F16, tag="pt")
                    for dcol in range(ND):
                        nc.tensor.transpose(ptH[:, dcol * IN:(dcol + 1) * IN],
                                            Hc[:, dcol, :], id16[0:IN, 0:IN])
                    ptHv = ptH[:, 0:ND * IN].rearrange("p (d k) -> p d k", d=ND)
                    nc.vector.tensor_copy(Hq[:, g, :, :], ptHv[:, :, 0:ND])
                    nc.vector.tensor_scalar_mul(Hm[:, g, :, :], ptHv[:, :, ND:IN],
                                                100.0)

            # ---- coriolis + rhs + Neumann solve (all groups fused) ---------
            prod = hp.tile([128, NG, ND, ND], F32, tag="prod", bufs=2)
            nc.vector.tensor_tensor(
                prod[:], Hq[:],
                qd_all[:].unsqueeze(2).broadcast_to((128, NG, ND, ND)), ALU.mult)
            cor = hp.tile([128, NG, ND], F32)
            nc.vector.tensor_reduce(cor[:].unsqueeze(3), prod[:], op=ALU.add, axis=AX.X)
            r = hp.tile([128, NG, ND], F32)
            nc.vector.scalar_tensor_tensor(r[:], cor[:], -1.0, gqT[:],
                                           ALU.mult, ALU.add)
            z = hp.tile([128, NG, ND], F32, tag="z", bufs=2)
            nc.vector.tensor_copy(z[:], r[:])
            for _ in range(3):
                pr = hp.tile([128, NG, ND, ND], F32, tag="prod", bufs=2)
                nc.vector.tensor_tensor(
                    pr[:], Hm[:],
                    z[:].unsqueeze(2).broadcast_to((128, NG, ND, ND)), ALU.mult)
                s_ = hp.tile([128, NG, ND], F32, tag="s", bufs=2)
                nc.vector.tensor_reduce(s_[:].unsqueeze(3), pr[:], op=ALU.add, axis=AX.X)
                zn = hp.tile([128, NG, ND], F32, tag="z", bufs=2)
                nc.vector.scalar_tensor_tensor(zn[:], s_[:], -1.0, r[:],
                                               ALU.mult, ALU.add)
                z = zn
            o = hp.tile([128, NG, ND], F32)
            nc.vector.tensor_scalar_mul(o[:], z[:], 100.0)
            for g in range(NG):
                nc.sync.dma_start(dout[g * 128:(g + 1) * 128, :], o[:, g, :])

    nc.compile()
    return nc


def kernel(**inputs):
    f16 = np.float16
    f32 = np.float32
    q = np.asarray(inputs["q"], f32)
    qdot = np.asarray(inputs["qdot"], f32)
    if "nc" not in _cache:
        _cache["nc"] = build_kernel()
    nc = _cache["nc"]
    W = [np.asarray(inputs[f"W{i}"], f32) for i in range(5)]
    X16 = np.ascontiguousarray(np.concatenate([q, qdot], axis=1)).astype(f16)
    base = {
        "wt0": np.ascontiguousarray(W[0].T).astype(f16),
        "wt1": np.ascontiguousarray(W[1].T).astype(f16),
        "wt2": np.ascontiguousarray(W[2].T).astype(f16),
        "wt3": np.ascontiguousarray(W[3].T).astype(f16),
        "wn1": np.ascontiguousarray(W[1]).astype(f16),
        "wn2": np.ascontiguousarray(W[2]).astype(f16),
        "wn3": np.ascontiguousarray(W[3]).astype(f16),
        "w0n": np.ascontiguousarray(W[0]).astype(f16),
        "w0qr": np.ascontiguousarray(
            np.repeat(W[0][:, ND:].astype(f16), T, axis=1)),
        "b0": inputs["b0"].reshape(H, 1).astype(f32),
        "b1": inputs["b1"].reshape(H, 1).astype(f32),
        "b2": inputs["b2"].reshape(H, 1).astype(f32),
        "b3": inputs["b3"].reshape(H, 1).astype(f32),
        "w4": np.ascontiguousarray(W[4].reshape(H, 1)).astype(f32),
        "id16": np.eye(128, dtype=f16),
        "id32": np.eye(128, dtype=f32),
    }
    in_maps = []
    for c in range(NC):
        m = dict(base)
        m["x16"] = X16[c * N:(c + 1) * N]
        m["qd32"] = np.ascontiguousarray(qdot[c * N:(c + 1) * N])
        in_maps.append(m)
    res = run_bass_kernel_spmd(nc, in_maps, core_ids=list(range(NC)),
                               trace=bool(os.environ.get("LNN_TRACE")))
    _cache["last"] = res
    out = np.concatenate([res.results[c]["qdd"] for c in range(NC)], axis=0)
    return out.astype(f32)


# revision 12
# speedup vs baseline: 1.2899x; 1.1802x over previous
# Lagrangian-NN qddot kernel for TRN2 (8 NeuronCores, data-parallel over batch).
#
# Math: scalar L(q,qdot) = MLP(24->256x4->1, softplus). Per sample:
#   M = d2L/dqdot2 + 0.01 I ; C = d2L/dqdot dq ; qddot = M^-1 (dL/dq - C qdot).
# Batched fwd+bwd gives grad; 12 qdot-direction forward-over-reverse tangents give
# H[:,12:] whose symmetry supplies both M and the Coriolis contraction.
# Everything except PSUM accumulation and the H/solve stage runs in fp16:
#  - DVE elementwise ops are emitted as scalar_tensor_tensor/tensor_scalar
#    (InstTensorScalarPtr) which support the 2x/4x DVE perf modes on all-SBUF
#    2-byte operands; PSUM results are staged to fp16 SBUF via Act/Pool copies.
#  - softplus/sigmoid composed from {exp,ln} + DVE max/adds (single ACT table):
#    Z = max(A,0) + ln(exp(-|A|)+1) ; S = exp(A - Z).
#  - M = 0.01(I + 100*Hqd) with ||100*Hqd|| <= 0.035, so the 12x12 solve is a
#    3-term Neumann series, fused over all 8 sample groups per core.
import os
import sys
import numpy as np

for p in ("/opt/trn_rl_repo", "/root/.axon_site/_ro/trn_rl_repo"):
    if p not in sys.path:
        sys.path.insert(0, p)

import concourse.bass as bass
import concourse.mybir as mybir
import concourse.tile as tile
from concourse import bacc
from concourse.bass_utils import run_bass_kernel_spmd

F32 = mybir.dt.float32
F16 = mybir.dt.float16
AF = mybir.ActivationFunctionType
ALU = mybir.AluOpType
AX = mybir.AxisListType

B, ND, H, NC = 8192, 12, 256, 8
N = B // NC          # samples per core
IN = 2 * ND          # 24
T = 64               # samples per tangent block
NT = N // T          # 16 blocks
NG = N // 128        # 8 groups of 128 samples
FD = ND * T          # 768 tangent free dim
CH = 512             # psum bank chunk (fp32 cols)
KT = H // 128        # 2 k-tiles per hidden dim

_cache = {}


def build_kernel():
    nc = bacc.Bacc("TRN2", target_bir_lowering=False)
    dx16 = nc.dram_tensor("x16", (N, IN), F16, kind="ExternalInput")
    dqd = nc.dram_tensor("qd32", (N, ND), F32, kind="ExternalInput")
    dwt0 = nc.dram_tensor("wt0", (IN, H), F16, kind="ExternalInput")
    dwt = {l: nc.dram_tensor(f"wt{l}", (H, H), F16, kind="ExternalInput")
           for l in (1, 2, 3)}
    dwn = {l: nc.dram_tensor(f"wn{l}", (H, H), F16, kind="ExternalInput")
           for l in (1, 2, 3)}
    dw0n = nc.dram_tensor("w0n", (H, IN), F16, kind="ExternalInput")
    dw0qr = nc.dram_tensor("w0qr", (H, FD), F16, kind="ExternalInput")
    dbs = [nc.dram_tensor(f"b{l}", (H, 1), F32, kind="ExternalInput") for l in range(4)]
    dw4 = nc.dram_tensor("w4", (H, 1), F32, kind="ExternalInput")
    did16 = nc.dram_tensor("id16", (128, 128), F16, kind="ExternalInput")
    did32 = nc.dram_tensor("id32", (128, 128), F32, kind="ExternalInput")
    dout = nc.dram_tensor("qdd", (N, ND), F32, kind="ExternalOutput")

    with tile.TileContext(nc) as tc:
        with tc.tile_pool(name="wp", bufs=1) as wp, \
             tc.tile_pool(name="ap", bufs=1) as ap, \
             tc.tile_pool(name="sc", bufs=2) as sc, \
             tc.tile_pool(name="tg", bufs=2) as tg, \
             tc.tile_pool(name="hp", bufs=1) as hp, \
             tc.tile_pool(name="psB", bufs=3, space="PSUM") as psB, \
             tc.tile_pool(name="psT", bufs=2, space="PSUM") as psT:

            # ---- weight / const loads --------------------------------------
            id16 = wp.tile([128, 128], F16)
            nc.sync.dma_start(id16[:], did16[:])
            id32 = wp.tile([128, 128], F32)
            nc.sync.dma_start(id32[:], did32[:])

            WT0 = wp.tile([IN, H], F16)
            nc.sync.dma_start(WT0[:], dwt0[:])

            def load2(dram, Fr, tag, dt=F16):
                parts = []
                for ki in range(KT):
                    t_ = wp.tile([128, Fr], dt, tag=f"{tag}{ki}")
                    nc.sync.dma_start(t_[:], dram[ki * 128:(ki + 1) * 128, :])
                    parts.append(t_)
                return parts

            WT = {l: load2(dwt[l], H, f"wt{l}_") for l in (1, 2, 3)}
            Wn = {l: load2(dwn[l], H, f"wn{l}_") for l in (1, 2, 3)}
            W0n = load2(dw0n, IN, "w0n_")
            W0qr = load2(dw0qr, FD, "w0qr_")
            bs = []
            for l in range(4):
                row = []
                for ki in range(KT):
                    t_ = wp.tile([128, 1], F32, tag=f"b{l}_{ki}")
                    nc.sync.dma_start(t_[:], dbs[l][ki * 128:(ki + 1) * 128, :])
                    row.append(t_)
                bs.append(row)
            w4t = []
            for ki in range(KT):
                t_ = wp.tile([128, 1], F32, tag=f"w4_{ki}")
                nc.sync.dma_start(t_[:], dw4[ki * 128:(ki + 1) * 128, :])
                w4t.append(t_)

            XS = hp.tile([128, NG, IN], F16)
            qd_all = hp.tile([128, NG, ND], F32)
            for g in range(NG):
                nc.sync.dma_start(XS[:, g, :], dx16[g * 128:(g + 1) * 128, :])
                nc.sync.dma_start(qd_all[:, g, :], dqd[g * 128:(g + 1) * 128, :])

            # ---- XT = X^T [24, N] fp16 -------------------------------------
            XT = hp.tile([IN, N], F16)
            for g in range(NG):
                pt = psT.tile([IN, 128], F16, tag="pt")
                nc.tensor.transpose(pt[:], XS[:, g, :], id16[:])
                nc.vector.tensor_copy(XT[:, g * 128:(g + 1) * 128], pt[:])

            def mm(ps_ap, lhsT_list, rhs_list, Fr):
                nk = len(lhsT_list)
                for c0 in range(0, Fr, CH):
                    ce = min(Fr, c0 + CH)
                    for ki in range(nk):
                        nc.tensor.matmul(ps_ap[:, c0:ce], lhsT_list[ki],
                                         rhs_list[ki][:, c0:ce],
                                         start=(ki == 0), stop=(ki == nk - 1))

            # ---- forward: Z chain + S (softplus/sigmoid via exp/ln) --------
            S = {}
            Zprev = [XT[:]]
            lhs0 = [WT0[:]]
            for l in range(4):
                Zcur = []
                for ot in range(KT):
                    ps = psB.tile([128, 1024], F32, tag="mm")
                    if l == 0:
                        mm(ps[:, 0:N], [lhs0[0][:, ot * 128:(ot + 1) * 128]],
                           Zprev, N)
                    else:
                        lts = [WT[l][ki][:, ot * 128:(ot + 1) * 128] for ki in range(KT)]
                        mm(ps[:, 0:N], lts, Zprev, N)
                    A16 = sc.tile([128, N], F16, tag="A16")
                    nc.scalar.activation(A16[:], ps[0:128, 0:N], AF.Identity,
                                         bias=bs[l][ot][:])
                    ab = sc.tile([128, N], F16, tag="t1")
                    nc.scalar.activation(ab[:], A16[:], AF.Abs)
                    ex = sc.tile([128, N], F16, tag="t2")
                    nc.scalar.activation(ex[:], ab[:], AF.Exp, scale=-1.0)
                    Ln = sc.tile([128, N], F16, tag="L")
                    nc.scalar.activation(Ln[:], ex[:], AF.Ln, bias=1.0)
                    rl = sc.tile([128, N], F16, tag="t1")
                    nc.vector.tensor_scalar_max(rl[:], A16[:], 0.0)
                    Z = sc.tile([128, N], F16, tag="Z", bufs=3)
                    nc.vector.tensor_add(Z[:], rl[:], Ln[:])
                    d = sc.tile([128, N], F16, tag="t2")
                    nc.vector.tensor_sub(d[:], A16[:], Z[:])
                    St = ap.tile([128, N], F16, tag=f"S{l}_{ot}")
                    nc.scalar.activation(St[:], d[:], AF.Exp)
                    S[(l, ot)] = St
                    Zcur.append(Z)
                Zprev = [z[:] for z in Zcur]

            # ---- D4 / c4 ----------------------------------------------------
            D4, c4 = [], []
            for ot in range(KT):
                Dt4 = ap.tile([128, N], F16, tag=f"D4_{ot}")
                nc.vector.tensor_scalar_mul(Dt4[:], S[(3, ot)][:], w4t[ot][:])
                D4.append(Dt4)
                OmS = sc.tile([128, N], F16, tag="OmS")
                nc.vector.tensor_scalar(OmS[:], S[(3, ot)][:], -1.0, 1.0,
                                        ALU.mult, ALU.add)
                tm = sc.tile([128, N], F16, tag="t1")
                nc.vector.tensor_mul(tm[:], OmS[:], S[(3, ot)][:])
                ct = ap.tile([128, N], F16, tag=f"c4_{ot}")
                nc.vector.tensor_scalar_mul(ct[:], tm[:], w4t[ot][:])
                c4.append(ct)

            # ---- backward D chain + F/E + g_q ------------------------------
            F = {}
            E1 = []
            Dprev = [d[:] for d in D4]
            for l in (2, 1, 0):
                Dcur = []
                for ot in range(KT):
                    ps = psB.tile([128, 1024], F32, tag="mm")
                    lts = [Wn[l + 1][ki][:, ot * 128:(ot + 1) * 128] for ki in range(KT)]
                    mm(ps[:, 0:N], lts, Dprev, N)
                    Ucp = sc.tile([128, N], F16, tag="Ucp")
                    nc.scalar.activation(Ucp[:], ps[0:128, 0:N], AF.Copy)
                    Dt = sc.tile([128, N], F16, tag="Dt", bufs=3)
                    nc.vector.tensor_mul(Dt[:], Ucp[:], S[(l, ot)][:])
                    if l > 0:
                        Ft = ap.tile([128, N], F16, tag=f"F{l}_{ot}")
                        nc.vector.tensor_sub(Ft[:], Ucp[:], Dt[:])
                        F[(l, ot)] = Ft
                    else:
                        OmS1 = sc.tile([128, N], F16, tag="OmS")
                        nc.vector.tensor_scalar(OmS1[:], S[(0, ot)][:], -1.0, 1.0,
                                                ALU.mult, ALU.add)
                        Et = ap.tile([128, N], F16, tag=f"E1_{ot}")
                        nc.vector.tensor_mul(Et[:], Dt[:], OmS1[:])
                        E1.append(Et)
                    Dcur.append(Dt)
                Dprev = [d[:] for d in Dcur]

            psG = psB.tile([128, 1024], F32, tag="mm")
            mm(psG[0:IN, 0:N], [W0n[ki][:] for ki in range(KT)], Dprev, N)
            Gcp = hp.tile([IN, N], F32)
            nc.scalar.activation(Gcp[:], psG[0:IN, 0:N], AF.Copy)

            # transposed g_q per group -> gqT [128, g, 12]
            gqT = hp.tile([128, NG, ND], F32)
            for g in range(NG):
                ptg = psT.tile([128, ND], F32, tag="pt")
                nc.tensor.transpose(ptg[:], Gcp[0:ND, g * 128:(g + 1) * 128],
                                    id32[0:ND, 0:ND])
                nc.vector.tensor_copy(gqT[:, g, :], ptg[:])

            # ---- tangent blocks --------------------------------------------
            Hq = hp.tile([128, NG, ND, ND], F32)   # H[x_j<12, qd_i] -> [p,g,i,j]
            Hm = hp.tile([128, NG, ND, ND], F32)   # 100*H[x_12+j, qd_i]
            Hc = None

            def bca(l, ot, sl):
                return S[(l, ot)][:, sl].unsqueeze(1).broadcast_to((128, ND, T))

            for b in range(NT):
                g, off = b // 2, (b % 2) * T
                sl = slice(b * T, (b + 1) * T)

                Zd1 = []
                for ot in range(KT):
                    z = tg.tile([128, ND, T], F16, tag=f"Zd1_{ot}")
                    w0v = W0qr[ot][:].rearrange("p (d t) -> p d t", d=ND)
                    nc.vector.tensor_mul(z[:], w0v, bca(0, ot, sl))
                    Zd1.append(z)

                def flat(ts):
                    return [t_[:].rearrange("p d t -> p (d t)") for t_ in ts]

                def tang_fwd(Zin, lW, Sl, Scoef, ztag, copy_eng):
                    outs = []
                    for ot in range(KT):
                        ps = psB.tile([128, 1024], F32, tag="mm")
                        lts = [lW[ki][:, ot * 128:(ot + 1) * 128] for ki in range(KT)]
                        mm(ps[:, 0:FD], lts, flat(Zin), FD)
                        cc = tg.tile([128, ND, T], F16, tag=f"c_{ot}", bufs=3)
                        ccf = cc[:].rearrange("p d t -> p (d t)")
                        nc.scalar.activation(ccf, ps[0:128, 0:FD], AF.Copy)
                        z = tg.tile([128, ND, T], F16, tag=f"{ztag}_{ot}")
                        nc.vector.tensor_mul(
                            z[:], cc[:],
                            Scoef[ot][:, sl].unsqueeze(1).broadcast_to((128, ND, T)))
                        outs.append(z)
                    return outs

                Zd2 = tang_fwd(Zd1, WT[1], None, [S[(1, 0)], S[(1, 1)]], "Zd2", "act")
                Zd3 = tang_fwd(Zd2, WT[2], None, [S[(2, 0)], S[(2, 1)]], "Zd3", "pool")
                Dd4 = tang_fwd(Zd3, WT[3], None, c4, "Dd4", "pool")

                def tang_bwd(Ddup, lW, Sl_l, Zdl, Fl, dtag, copy_eng):
                    outs = []
                    for ot in range(KT):
                        ps = psB.tile([128, 1024], F32, tag="mm")
                        lts = [lW[ki][:, ot * 128:(ot + 1) * 128] for ki in range(KT)]
                        mm(ps[:, 0:FD], lts, flat(Ddup), FD)
                        u = tg.tile([128, ND, T], F16, tag=f"u_{ot}")
                        if copy_eng == "act":
                            y = tg.tile([128, ND, T], F16, tag=f"y_{ot}", bufs=3)
                            nc.scalar.activation(y[:].rearrange("p d t -> p (d t)"),
                                                 ps[0:128, 0:FD], AF.Copy)
                            nc.vector.tensor_mul(u[:], y[:], bca(Sl_l, ot, sl))
                        else:
                            psv = ps[0:128, 0:FD].rearrange("p (d t) -> p d t", d=ND)
                            nc.vector.tensor_mul(u[:], psv, bca(Sl_l, ot, sl))
                        t_ = tg.tile([128, ND, T], F16, tag=f"t_{ot}")
                        nc.vector.tensor_mul(
                            t_[:], Zdl[ot][:],
                            Fl[ot][:, sl].unsqueeze(1).broadcast_to((128, ND, T)))
                        dd = tg.tile([128, ND, T], F16, tag=f"{dtag}_{ot}")
                        nc.gpsimd.tensor_add(dd[:].rearrange("p d t -> p (d t)"),
                                             u[:].rearrange("p d t -> p (d t)"),
                                             t_[:].rearrange("p d t -> p (d t)"))
                        outs.append(dd)
                    return outs

                Dd3 = tang_bwd(Dd4, Wn[3], 2, Zd3, [F[(2, 0)], F[(2, 1)]], "DdA", "act")
                Dd2 = tang_bwd(Dd3, Wn[2], 1, Zd2, [F[(1, 0)], F[(1, 1)]], "DdB", "direct")

                # layer 1: t = E1 * W0qr
                Dd1 = []
                for ot in range(KT):
                    ps = psB.tile([128, 1024], F32, tag="mm")
                    lts = [Wn[1][ki][:, ot * 128:(ot + 1) * 128] for ki in range(KT)]
                    mm(ps[:, 0:FD], lts, flat(Dd2), FD)
                    y = tg.tile([128, ND, T], F16, tag=f"y_{ot}", bufs=3)
                    nc.scalar.activation(y[:].rearrange("p d t -> p (d t)"),
                                         ps[0:128, 0:FD], AF.Copy)
                    u = tg.tile([128, ND, T], F16, tag=f"u_{ot}")
                    nc.vector.tensor_mul(u[:], y[:], bca(0, ot, sl))
                    t_ = tg.tile([128, ND, T], F16, tag=f"t_{ot}")
                    w0v = W0qr[ot][:].rearrange("p (d t) -> p d t", d=ND)
                    nc.vector.tensor_mul(
                        t_[:], w0v,
                        E1[ot][:, sl].unsqueeze(1).broadcast_to((128, ND, T)))
                    dd = tg.tile([128, ND, T], F16, tag=f"DdA_{ot}")
                    if ot == 0:
                        nc.gpsimd.tensor_add(dd[:].rearrange("p d t -> p (d t)"),
                                             u[:].rearrange("p d t -> p (d t)"),
                                             t_[:].rearrange("p d t -> p (d t)"))
                    else:
                        nc.vector.tensor_add(dd[:], u[:], t_[:])
                    Dd1.append(dd)

                psH = psB.tile([128, 1024], F32, tag="mm")
                mm(psH[0:IN, 0:FD], [W0n[ki][:] for ki in range(KT)], flat(Dd1), FD)
                if off == 0:
                    Hc = hp.tile([IN, ND, 128], F16, tag="Hc", bufs=2)
                nc.scalar.activation(Hc[:, :, off:off + T],
                                     psH[0:IN, 0:FD].rearrange("p (d t) -> p d t", d=ND),
                                     AF.Copy)

                if off == T:
                    ptH = psT.tile([128, 288], F16, tag="pt")
                    for dcol in range(ND):
                        nc.tensor.transpose(ptH[:, dcol * IN:(dcol + 1) * IN],
                                            Hc[:, dcol, :], id16[0:IN, 0:IN])
                    ptHv = ptH[:, 0:ND * IN].rearrange("p (d k) -> p d k", d=ND)
                    nc.vector.tensor_copy(Hq[:, g, :, :], ptHv[:, :, 0:ND])
                    nc.vector.tensor_scalar_mul(Hm[:, g, :, :], ptHv[:, :, ND:IN],
                                                100.0)

            # ---- coriolis + rhs + Neumann solve (all groups fused) ---------
            prod = hp.tile([128, NG, ND, ND], F32, tag="prod", bufs=2)
            nc.vector.tensor_tensor(
                prod[:], Hq[:],
                qd_all[:].unsqueeze(2).broadcast_to((128, NG, ND, ND)), ALU.mult)
            cor = hp.tile([128, NG, ND], F32)
            nc.vector.tensor_reduce(cor[:].unsqueeze(3), prod[:], op=ALU.add, axis=AX.X)
            r = hp.tile([128, NG, ND], F32)
            nc.vector.scalar_tensor_tensor(r[:], cor[:], -1.0, gqT[:],
                                           ALU.mult, ALU.add)
            z = hp.tile([128, NG, ND], F32, tag="z", bufs=2)
            nc.vector.tensor_copy(z[:], r[:])
            for _ in range(3):
                pr = hp.tile([128, NG, ND, ND], F32, tag="prod", bufs=2)
                nc.vector.tensor_tensor(
                    pr[:], Hm[:],
                    z[:].unsqueeze(2).broadcast_to((128, NG, ND, ND)), ALU.mult)
                s_ = hp.tile([128, NG, ND], F32, tag="s", bufs=2)
                nc.vector.tensor_reduce(s_[:].unsqueeze(3), pr[:], op=ALU.add, axis=AX.X)
                zn = hp.tile([128, NG, ND], F32, tag="z", bufs=2)
                nc.vector.scalar_tensor_tensor(zn[:], s_[:], -1.0, r[:],
                                               ALU.mult, ALU.add)
                z = zn
            o = hp.tile([128, NG, ND], F32)
            nc.vector.tensor_scalar_mul(o[:], z[:], 100.0)
            for g in range(NG):
                nc.sync.dma_start(dout[g * 128:(g + 1) * 128, :], o[:, g, :])

    nc.compile()
    return nc


def kernel(**inputs):
    f16 = np.float16
    f32 = np.float32
    q = np.asarray(inputs["q"], f32)
    qdot = np.asarray(inputs["qdot"], f32)
    if "nc" not in _cache:
        _cache["nc"] = build_kernel()
    nc = _cache["nc"]
    W = [np.asarray(inputs[f"W{i}"], f32) for i in range(5)]
    X16 = np.ascontiguousarray(np.concatenate([q, qdot], axis=1)).astype(f16)
    base = {
        "wt0": np.ascontiguousarray(W[0].T).astype(f16),
        "wt1": np.ascontiguousarray(W[1].T).astype(f16),
        "wt2": np.ascontiguousarray(W[2].T).astype(f16),
        "wt3": np.ascontiguousarray(W[3].T).astype(f16),
        "wn1": np.ascontiguousarray(W[1]).astype(f16),
        "wn2": np.ascontiguousarray(W[2]).astype(f16),
        "wn3": np.ascontiguousarray(W[3]).astype(f16),
        "w0n": np.ascontiguousarray(W[0]).astype(f16),
        "w0qr": np.ascontiguousarray(
            np.repeat(W[0][:, ND:].astype(f16), T, axis=1)),
        "b0": inputs["b0"].reshape(H, 1).astype(f32),
        "b1": inputs["b1"].reshape(H, 1).astype(f32),
        "b2": inputs["b2"].reshape(H, 1).astype(f32),
        "b3": inputs["b3"].reshape(H, 1).astype(f32),
        "w4": np.ascontiguousarray(W[4].reshape(H, 1)).astype(f32),
        "id16": np.eye(128, dtype=f16),
        "id32": np.eye(128, dtype=f32),
    }
    in_maps = []
    for c in range(NC):
        m = dict(base)
        m["x16"] = X16[c * N:(c + 1) * N]
        m["qd32"] = np.ascontiguousarray(qdot[c * N:(c + 1) * N])
        in_maps.append(m)
    res = run_bass_kernel_spmd(nc, in_maps, core_ids=list(range(NC)),
                               trace=bool(os.environ.get("LNN_TRACE")))
    _cache["last"] = res
    out = np.concatenate([res.results[c]["qdd"] for c in range(NC)], axis=0)
    return out.astype(f32)


# revision 14
# speedup vs baseline: 1.8248x; 1.4147x over previous
# Lagrangian-NN qddot kernel for TRN2 (8 NeuronCores, data-parallel over batch).
#
# Math: scalar L(q,qdot) = MLP(24->256x4->1, softplus). Per sample:
#   M = d2L/dqdot2 + 0.01 I ; C = d2L/dqdot dq ; qddot = M^-1 (dL/dq - C qdot).
# Batched fwd+bwd gives grad; 12 qdot-direction forward-over-reverse tangents give
# H[:,12:] whose symmetry supplies both M and the Coriolis contraction.
# Everything except PSUM accumulation and the H/solve stage runs in fp16:
#  - DVE elementwise ops are emitted as scalar_tensor_tensor/tensor_scalar
#    (InstTensorScalarPtr) which support the 2x/4x DVE perf modes on all-SBUF
#    2-byte operands; PSUM results are staged to fp16 SBUF via Act/Pool copies.
#  - softplus/sigmoid composed from {exp,ln} + DVE max/adds (single ACT table):
#    Z = max(A,0) + ln(exp(-|A|)+1) ; S = exp(A - Z).
#  - M = 0.01(I + 100*Hqd) with ||100*Hqd|| <= 0.035, so the 12x12 solve is a
#    3-term Neumann series, fused over all 8 sample groups per core.
import os
import sys
import numpy as np

for p in ("/opt/trn_rl_repo", "/root/.axon_site/_ro/trn_rl_repo"):
    if p not in sys.path:
        sys.path.insert(0, p)

import concourse.bass as bass
import concourse.mybir as mybir
import concourse.tile as tile
from concourse import bacc
from concourse.bass_utils import run_bass_kernel_spmd

F32 = mybir.dt.float32
F16 = mybir.dt.float16
AF = mybir.ActivationFunctionType
ALU = mybir.AluOpType
AX = mybir.AxisListType

B, ND, H, NC = 8192, 12, 256, 8
N = B // NC          # samples per core
IN = 2 * ND          # 24
T = 64               # samples per tangent block
NT = N // T          # 16 blocks
NG = N // 128        # 8 groups of 128 samples
FD = ND * T          # 768 tangent free dim
CH = 512             # psum bank chunk (fp32 cols)
KT = H // 128        # 2 k-tiles per hidden dim

_cache = {}


def build_kernel():
    nc = bacc.Bacc("TRN2", target_bir_lowering=False)
    dx16 = nc.dram_tensor("x16", (N, IN), F16, kind="ExternalInput")
    dqd = nc.dram_tensor("qd32", (N, ND), F32, kind="ExternalInput")
    dwt0 = nc.dram_tensor("wt0", (IN, H), F16, kind="ExternalInput")
    dwt = {l: nc.dram_tensor(f"wt{l}", (H, H), F16, kind="ExternalInput")
           for l in (1, 2, 3)}
    dwn = {l: nc.dram_tensor(f"wn{l}", (H, H), F16, kind="ExternalInput")
           for l in (1, 2, 3)}
    dw0n = nc.dram_tensor("w0n", (H, IN), F16, kind="ExternalInput")
    dw0qr = nc.dram_tensor("w0qr", (H, FD), F16, kind="ExternalInput")
    dbs = [nc.dram_tensor(f"b{l}", (H, 1), F32, kind="ExternalInput") for l in range(4)]
    dw4 = nc.dram_tensor("w4", (H, 1), F32, kind="ExternalInput")
    did16 = nc.dram_tensor("id16", (128, 128), F16, kind="ExternalInput")
    did32 = nc.dram_tensor("id32", (128, 128), F32, kind="ExternalInput")
    dout = nc.dram_tensor("qdd", (N, ND), F32, kind="ExternalOutput")

    with tile.TileContext(nc) as tc:
        with tc.tile_pool(name="wp", bufs=1) as wp, \
             tc.tile_pool(name="ap", bufs=1) as ap, \
             tc.tile_pool(name="sc", bufs=2) as sc, \
             tc.tile_pool(name="tg", bufs=2) as tg, \
             tc.tile_pool(name="hp", bufs=1) as hp, \
             tc.tile_pool(name="psB", bufs=3, space="PSUM") as psB, \
             tc.tile_pool(name="psT", bufs=2, space="PSUM") as psT:

            # ---- weight / const loads --------------------------------------
            id16 = wp.tile([128, 128], F16)
            nc.sync.dma_start(id16[:], did16[:])
            id32 = wp.tile([128, 128], F32)
            nc.sync.dma_start(id32[:], did32[:])

            WT0 = wp.tile([IN, H], F16)
            nc.sync.dma_start(WT0[:], dwt0[:])

            def load2(dram, Fr, tag, dt=F16):
                parts = []
                for ki in range(KT):
                    t_ = wp.tile([128, Fr], dt, tag=f"{tag}{ki}")
                    nc.sync.dma_start(t_[:], dram[ki * 128:(ki + 1) * 128, :])
                    parts.append(t_)
                return parts

            WT = {l: load2(dwt[l], H, f"wt{l}_") for l in (1, 2, 3)}
            Wn = {l: load2(dwn[l], H, f"wn{l}_") for l in (1, 2, 3)}
            W0n = load2(dw0n, IN, "w0n_")
            W0qr = load2(dw0qr, FD, "w0qr_")
            bs = []
            for l in range(4):
                row = []
                for ki in range(KT):
                    t_ = wp.tile([128, 1], F32, tag=f"b{l}_{ki}")
                    nc.sync.dma_start(t_[:], dbs[l][ki * 128:(ki + 1) * 128, :])
                    row.append(t_)
                bs.append(row)
            w4t = []
            for ki in range(KT):
                t_ = wp.tile([128, 1], F32, tag=f"w4_{ki}")
                nc.sync.dma_start(t_[:], dw4[ki * 128:(ki + 1) * 128, :])
                w4t.append(t_)

            XS = hp.tile([128, NG, IN], F16)
            qd_all = hp.tile([128, NG, ND], F32)
            for g in range(NG):
                nc.sync.dma_start(XS[:, g, :], dx16[g * 128:(g + 1) * 128, :])
                nc.sync.dma_start(qd_all[:, g, :], dqd[g * 128:(g + 1) * 128, :])

            # ---- XT = X^T [24, N] fp16 -------------------------------------
            XT = hp.tile([IN, N], F16)
            for g in range(NG):
                pt = psT.tile([IN, 128], F16, tag="pt")
                nc.tensor.transpose(pt[:], XS[:, g, :], id16[:])
                nc.vector.tensor_copy(XT[:, g * 128:(g + 1) * 128], pt[:])

            def mm(ps_ap, lhsT_list, rhs_list, Fr):
                nk = len(lhsT_list)
                for c0 in range(0, Fr, CH):
                    ce = min(Fr, c0 + CH)
                    for ki in range(nk):
                        nc.tensor.matmul(ps_ap[:, c0:ce], lhsT_list[ki],
                                         rhs_list[ki][:, c0:ce],
                                         start=(ki == 0), stop=(ki == nk - 1))

            # ---- forward: Z chain + S (softplus/sigmoid via exp/ln) --------
            S = {}
            Zprev = [XT[:]]
            lhs0 = [WT0[:]]
            for l in range(4):
                Zcur = []
                for ot in range(KT):
                    ps = psB.tile([128, 1024], F32, tag="mm")
                    if l == 0:
                        mm(ps[:, 0:N], [lhs0[0][:, ot * 128:(ot + 1) * 128]],
                           Zprev, N)
                    else:
                        lts = [WT[l][ki][:, ot * 128:(ot + 1) * 128] for ki in range(KT)]
                        mm(ps[:, 0:N], lts, Zprev, N)
                    A16 = sc.tile([128, N], F16, tag="A16")
                    nc.scalar.activation(A16[:], ps[0:128, 0:N], AF.Identity,
                                         bias=bs[l][ot][:])
                    ab = sc.tile([128, N], F16, tag="t1")
                    nc.scalar.activation(ab[:], A16[:], AF.Abs)
                    ex = sc.tile([128, N], F16, tag="t2")
                    nc.scalar.activation(ex[:], ab[:], AF.Exp, scale=-1.0)
                    Ln = sc.tile([128, N], F16, tag="L")
                    nc.scalar.activation(Ln[:], ex[:], AF.Ln, bias=1.0)
                    rl = sc.tile([128, N], F16, tag="t1")
                    nc.vector.tensor_scalar_max(rl[:], A16[:], 0.0)
                    Z = sc.tile([128, N], F16, tag="Z", bufs=3)
                    nc.vector.tensor_add(Z[:], rl[:], Ln[:])
                    d = sc.tile([128, N], F16, tag="t2")
                    nc.vector.tensor_sub(d[:], A16[:], Z[:])
                    St = ap.tile([128, N], F16, tag=f"S{l}_{ot}")
                    nc.scalar.activation(St[:], d[:], AF.Exp)
                    S[(l, ot)] = St
                    Zcur.append(Z)
                Zprev = [z[:] for z in Zcur]

            # ---- D4 / c4 ----------------------------------------------------
            D4, c4 = [], []
            for ot in range(KT):
                Dt4 = ap.tile([128, N], F16, tag=f"D4_{ot}")
                nc.vector.tensor_scalar_mul(Dt4[:], S[(3, ot)][:], w4t[ot][:])
                D4.append(Dt4)
                OmS = sc.tile([128, N], F16, tag="OmS")
                nc.vector.tensor_scalar(OmS[:], S[(3, ot)][:], -1.0, 1.0,
                                        ALU.mult, ALU.add)
                tm = sc.tile([128, N], F16, tag="t1")
                nc.vector.tensor_mul(tm[:], OmS[:], S[(3, ot)][:])
                ct = ap.tile([128, N], F16, tag=f"c4_{ot}")
                nc.vector.tensor_scalar_mul(ct[:], tm[:], w4t[ot][:])
                c4.append(ct)

            # ---- backward D chain + F/E + g_q ------------------------------
            F = {}
            E1 = []
            Dprev = [d[:] for d in D4]
            for l in (2, 1, 0):
                Dcur = []
                for ot in range(KT):
                    ps = psB.tile([128, 1024], F32, tag="mm")
                    lts = [Wn[l + 1][ki][:, ot * 128:(ot + 1) * 128] for ki in range(KT)]
                    mm(ps[:, 0:N], lts, Dprev, N)
                    Ucp = sc.tile([128, N], F16, tag="Ucp")
                    nc.scalar.activation(Ucp[:], ps[0:128, 0:N], AF.Copy)
                    Dt = sc.tile([128, N], F16, tag="Dt", bufs=3)
                    nc.vector.tensor_mul(Dt[:], Ucp[:], S[(l, ot)][:])
                    if l > 0:
                        Ft = ap.tile([128, N], F16, tag=f"F{l}_{ot}")
                        nc.vector.tensor_sub(Ft[:], Ucp[:], Dt[:])
                        F[(l, ot)] = Ft
                    else:
                        OmS1 = sc.tile([128, N], F16, tag="OmS")
                        nc.vector.tensor_scalar(OmS1[:], S[(0, ot)][:], -1.0, 1.0,
                                                ALU.mult, ALU.add)
                        Et = ap.tile([128, N], F16, tag=f"E1_{ot}")
                        nc.vector.tensor_mul(Et[:], Dt[:], OmS1[:])
                        E1.append(Et)
                    Dcur.append(Dt)
                Dprev = [d[:] for d in Dcur]

            psG = psB.tile([128, 1024], F32, tag="mm")
            mm(psG[0:IN, 0:N], [W0n[ki][:] for ki in range(KT)], Dprev, N)
            Gcp = hp.tile([IN, N], F32)
            nc.scalar.activation(Gcp[:], psG[0:IN, 0:N], AF.Copy)

            # transposed g_q per group -> gqT [128, g, 12]
            gqT = hp.tile([128, NG, ND], F32)
            for g in range(NG):
                ptg = psT.tile([128, ND], F32, tag="pt")
                nc.tensor.transpose(ptg[:], Gcp[0:ND, g * 128:(g + 1) * 128],
                                    id32[0:ND, 0:ND])
                nc.vector.tensor_copy(gqT[:, g, :], ptg[:])

            # ---- tangent blocks --------------------------------------------
            Hq = hp.tile([128, NG, ND, ND], F32)   # H[x_j<12, qd_i] -> [p,g,i,j]
            Hm = hp.tile([128, NG, ND, ND], F32)   # 100*H[x_12+j, qd_i]
            Hc = None

            def bca(l, ot, sl):
                return S[(l, ot)][:, sl].unsqueeze(1).broadcast_to((128, ND, T))

            def flat(ts):
                return [t_[:].rearrange("p d t -> p (d t)") for t_ in ts]

            def make_steps(b, Hc_ref):
                """Return the tangent-chain step closures for block b.

                Blocks are emitted pairwise-interleaved so every engine queue
                alternates between two independent chains (fills the bubbles a
                single serial chain leaves on the other engines)."""
                i = b & 1
                off = i * T
                g = b // 2
                sl = slice(b * T, (b + 1) * T)
                st = {}

                def w0v(ot):
                    return W0qr[ot][:].rearrange("p (d t) -> p d t", d=ND)

                def tgt(tagbase, **kw):
                    kw.setdefault("bufs", 1)
                    return tg.tile([128, ND, T], F16, tag=f"{tagbase}_{i}",
                                   name=tagbase, **kw)

                def s_zd1():
                    st["Zd1"] = []
                    for ot in range(KT):
                        z = tgt(f"Zd1_{ot}")
                        nc.vector.tensor_mul(z[:], w0v(ot), bca(0, ot, sl))
                        st["Zd1"].append(z)

                def mk_mm(src_key, lW, dst_key, rows=128):
                    def s_mm():
                        pss = []
                        for ot in range(KT if rows == 128 else 1):
                            ps = psB.tile([128, 1024], F32, tag="mm", name="ps")
                            if rows == 128:
                                lts = [lW[ki][:, ot * 128:(ot + 1) * 128]
                                       for ki in range(KT)]
                            else:
                                lts = [lW[ki][:] for ki in range(KT)]
                            mm(ps[0:rows, 0:FD], lts, flat(st[src_key]), FD)
                            pss.append(ps)
                        st[dst_key] = pss
                    return s_mm

                def mk_copy(ps_key, dst_key, dtag):
                    def s_copy():
                        st[dst_key] = []
                        for ot in range(KT):
                            cc = tgt(f"{dtag}_{ot}")
                            nc.scalar.activation(cc[:].rearrange("p d t -> p (d t)"),
                                                 st[ps_key][ot][0:128, 0:FD], AF.Copy)
                            st[dst_key].append(cc)
                    return s_copy

                def mk_mul(in_key, coefs, dst_key, dtag=None):
                    def s_mul():
                        st[dst_key] = []
                        for ot in range(KT):
                            z = tgt(f"{dtag or dst_key}_{ot}")
                            nc.vector.tensor_mul(
                                z[:], st[in_key][ot][:],
                                coefs[ot][:, sl].unsqueeze(1).broadcast_to(
                                    (128, ND, T)))
                            st[dst_key].append(z)
                    return s_mul

                def mk_umul_direct(ps_key, coefs, dst_key):
                    def s_mul():
                        st[dst_key] = []
                        for ot in range(KT):
                            z = tgt(f"u2_{ot}")
                            psv = st[ps_key][ot][0:128, 0:FD].rearrange(
                                "p (d t) -> p d t", d=ND)
                            nc.vector.tensor_mul(
                                z[:], psv,
                                coefs[ot][:, sl].unsqueeze(1).broadcast_to(
                                    (128, ND, T)))
                            st[dst_key].append(z)
                    return s_mul

                def mk_tmul(zd_key, coefs, dst_key, use_w0=False):
                    def s_mul():
                        st[dst_key] = []
                        for ot in range(KT):
                            z = tgt(f"t_{ot}")
                            src = w0v(ot) if use_w0 else st[zd_key][ot][:]
                            nc.vector.tensor_mul(
                                z[:], src,
                                coefs[ot][:, sl].unsqueeze(1).broadcast_to(
                                    (128, ND, T)))
                            st[dst_key].append(z)
                    return s_mul

                def mk_add(u_key, t_key, dst_key, dtag, pool_mask=3):
                    def s_add():
                        st[dst_key] = []
                        for ot in range(KT):
                            dd = tgt(f"{dtag}_{ot}")
                            if (pool_mask >> ot) & 1:
                                nc.gpsimd.tensor_add(
                                    dd[:].rearrange("p d t -> p (d t)"),
                                    st[u_key][ot][:].rearrange("p d t -> p (d t)"),
                                    st[t_key][ot][:].rearrange("p d t -> p (d t)"))
                            else:
                                nc.vector.tensor_add(dd[:], st[u_key][ot][:],
                                                     st[t_key][ot][:])
                            st[dst_key].append(dd)
                    return s_add

                def s_hc():
                    if off == 0:
                        Hc_ref[0] = hp.tile([IN, ND, 128], F16, tag="Hc", bufs=2,
                                            name="Hc")
                    nc.scalar.activation(
                        Hc_ref[0][:, :, off:off + T],
                        st["psH"][0][0:IN, 0:FD].rearrange("p (d t) -> p d t", d=ND),
                        AF.Copy)

                def s_hstage():
                    if off != T:
                        return
                    ptH = psT.tile([128, 288], F16, tag="pt", name="ptH")
                    for dcol in range(ND):
                        nc.tensor.transpose(ptH[:, dcol * IN:(dcol + 1) * IN],
                                            Hc_ref[0][:, dcol, :], id16[0:IN, 0:IN])
                    ptHv = ptH[:, 0:ND * IN].rearrange("p (d k) -> p d k", d=ND)
                    nc.vector.tensor_copy(Hq[:, g, :, :], ptHv[:, :, 0:ND])
                    nc.vector.tensor_scalar_mul(Hm[:, g, :, :], ptHv[:, :, ND:IN],
                                                100.0)

                S2c = [S[(1, 0)], S[(1, 1)]]
                S3c = [S[(2, 0)], S[(2, 1)]]
                F3c = [F[(2, 0)], F[(2, 1)]]
                F2c = [F[(1, 0)], F[(1, 1)]]
                S1c = [S[(0, 0)], S[(0, 1)]]
                return [
                    s_zd1,
                    mk_mm("Zd1", WT[1], "psA"),
                    mk_copy("psA", "c2", "c"),
                    mk_mul("c2", S2c, "Zd2"),
                    mk_mm("Zd2", WT[2], "psB"),
                    mk_copy("psB", "c3", "c"),
                    mk_mul("c3", S3c, "Zd3"),
                    mk_mm("Zd3", WT[3], "psC"),
                    mk_copy("psC", "cY", "c"),
                    mk_mul("cY", c4, "Dd4"),
                    mk_mm("Dd4", Wn[3], "psY3"),
                    mk_copy("psY3", "y3", "y"),
                    mk_mul("y3", S3c, "u3", dtag="u"),
                    mk_tmul("Zd3", F3c, "t3"),
                    mk_add("u3", "t3", "Dd3", "DdA"),
                    mk_mm("Dd3", Wn[2], "psY2"),
                    mk_umul_direct("psY2", S2c, "u2"),
                    mk_tmul("Zd2", F2c, "t2"),
                    mk_add("u2", "t2", "Dd2", "DdB"),
                    mk_mm("Dd2", Wn[1], "psY1"),
                    mk_copy("psY1", "y1", "y"),
                    mk_mul("y1", S1c, "u1", dtag="u"),
                    mk_tmul(None, E1, "t1", use_w0=True),
                    mk_add("u1", "t1", "Dd1", "DdA", pool_mask=1),
                    mk_mm("Dd1", W0n, "psH", rows=IN),
                    s_hc,
                    s_hstage,
                ]

            for pair in range(NT // 2):
                Hc_ref = [None]
                steps0 = make_steps(2 * pair, Hc_ref)
                steps1 = make_steps(2 * pair + 1, Hc_ref)
                for s0, s1 in zip(steps0, steps1):
                    s0()
                    s1()

            # ---- coriolis + rhs + Neumann solve (all groups fused) ---------
            prod = hp.tile([128, NG, ND, ND], F32, tag="prod", bufs=2)
            nc.vector.tensor_tensor(
                prod[:], Hq[:],
                qd_all[:].unsqueeze(2).broadcast_to((128, NG, ND, ND)), ALU.mult)
            cor = hp.tile([128, NG, ND], F32)
            nc.vector.tensor_reduce(cor[:].unsqueeze(3), prod[:], op=ALU.add, axis=AX.X)
            r = hp.tile([128, NG, ND], F32)
            nc.vector.scalar_tensor_tensor(r[:], cor[:], -1.0, gqT[:],
                                           ALU.mult, ALU.add)
            z = hp.tile([128, NG, ND], F32, tag="z", bufs=2)
            nc.vector.tensor_copy(z[:], r[:])
            for _ in range(3):
                pr = hp.tile([128, NG, ND, ND], F32, tag="prod", bufs=2)
                nc.vector.tensor_tensor(
                    pr[:], Hm[:],
                    z[:].unsqueeze(2).broadcast_to((128, NG, ND, ND)), ALU.mult)
                s_ = hp.tile([128, NG, ND], F32, tag="s", bufs=2)
                nc.vector.tensor_reduce(s_[:].unsqueeze(3), pr[:], op=ALU.add, axis=AX.X)
                zn = hp.tile([128, NG, ND], F32, tag="z", bufs=2)
                nc.vector.scalar_tensor_tensor(zn[:], s_[:], -1.0, r[:],
                                               ALU.mult, ALU.add)
                z = zn
            o = hp.tile([128, NG, ND], F32)
            nc.vector.tensor_scalar_mul(o[:], z[:], 100.0)
            for g in range(NG):
                nc.sync.dma_start(dout[g * 128:(g + 1) * 128, :], o[:, g, :])

    nc.compile()
    return nc


def kernel(**inputs):
    f16 = np.float16
    f32 = np.float32
    q = np.asarray(inputs["q"], f32)
    qdot = np.asarray(inputs["qdot"], f32)
    if "nc" not in _cache:
        _cache["nc"] = build_kernel()
    nc = _cache["nc"]
    W = [np.asarray(inputs[f"W{i}"], f32) for i in range(5)]
    X16 = np.ascontiguousarray(np.concatenate([q, qdot], axis=1)).astype(f16)
    base = {
        "wt0": np.ascontiguousarray(W[0].T).astype(f16),
        "wt1": np.ascontiguousarray(W[1].T).astype(f16),
        "wt2": np.ascontiguousarray(W[2].T).astype(f16),
        "wt3": np.ascontiguousarray(W[3].T).astype(f16),
        "wn1": np.ascontiguousarray(W[1]).astype(f16),
        "wn2": np.ascontiguousarray(W[2]).astype(f16),
        "wn3": np.ascontiguousarray(W[3]).astype(f16),
        "w0n": np.ascontiguousarray(W[0]).astype(f16),
        "w0qr": np.ascontiguousarray(
            np.repeat(W[0][:, ND:].astype(f16), T, axis=1)),
        "b0": inputs["b0"].reshape(H, 1).astype(f32),
        "b1": inputs["b1"].reshape(H, 1).astype(f32),
        "b2": inputs["b2"].reshape(H, 1).astype(f32),
        "b3": inputs["b3"].reshape(H, 1).astype(f32),
        "w4": np.ascontiguousarray(W[4].reshape(H, 1)).astype(f32),
        "id16": np.eye(128, dtype=f16),
        "id32": np.eye(128, dtype=f32),
    }
    in_maps = []
    for c in range(NC):
        m = dict(base)
        m["x16"] = X16[c * N:(c + 1) * N]
        m["qd32"] = np.ascontiguousarray(qdot[c * N:(c + 1) * N])
        in_maps.append(m)
    res = run_bass_kernel_spmd(nc, in_maps, core_ids=list(range(NC)),
                               trace=bool(os.environ.get("LNN_TRACE")))
    _cache["last"] = res
    out = np.concatenate([res.results[c]["qdd"] for c in range(NC)], axis=0)
    return out.astype(f32)
